# revision 10
# baseline (speedup 1.0000x reference)
"""Trainium2 Bass kernel for nn_CustomConv2d: 3x3 conv, B=16, Cin=Cout=128, H=W=64.

Strategy (v3):
  - Data-parallel over batch: 8 NeuronCores x 2 images each; the (128,128,9)
    weight is replicated (host pre-transposes to [cin, tap, cout] so tap k is
    a contiguous [cin, cout] stationary-operand slice).
  - fp32r matmuls (TF32-like, 1 cycle/row; bf16 moving operands measured
    SLOWER, ~250ns vs ~231ns per 512-row matmul).  Host pre-rounds inputs to
    fp32r so on-device numerics are deterministic.
  - Per image the feature map lives in SBUF as a 66x66 zero-padded plane
    (host-prepadded => every DMA is contiguous per partition).
  - Conv = 9 accumulating PE matmuls per 8-row output block (contraction over
    Cin=128 on the partition dim).
  - DMA plan built around the measured fixed costs per DMA instruction
    (~0.6us sequencer config + ~0.63us on the GLOBAL HWDGE generator +
    ~0.65us DGE->ring delay + 0.9us completion-semaphore propagation):
    few instructions, first chunks minimal, split across the two HWDGE
    engines (sync=SP carries img0 x chunks; scalar=Activation carries w in
    3 tap-groups then img1 chunks then per-block output DMAs).
  - PE warm-up: HAM un-throttles the PE clock only after ~2.5-3.4us of
    sustained array activity AND re-throttles after an idle window, so the
    warmup (bf16 junk matmuls on a zeroed tile, memset on the otherwise-idle
    vector engine) is sized to end exactly when the first conv data lands.
  - Tail: final block copied in halves with the two DMAs on sync+scalar so
    the kernel-exit drain starts as soon as possible.
"""

import numpy as np

import concourse.bass as bass  # noqa: F401  (registers bass types)
import concourse.tile as tile
import concourse.mybir as mybir
from concourse import bacc, bass_utils

F32 = mybir.dt.float32
F32R = mybir.dt.float32r
BF16 = mybir.dt.bfloat16

B, CIN, COUT, KK, H, W = 16, 128, 128, 3, 64, 64
NCORES = 8
BPC = B // NCORES  # images per core
HW = H * W         # 4096
PW = W + 2         # padded row length (66)
PH = H + 2         # padded rows (66)
XLEN = PH * PW     # 4356
ROWBLK = 8         # output rows per PSUM block (8*64=512 = one fp32 PSUM bank)
NBLK = H // ROWBLK # 8 blocks per image

WARMN = 8          # warmup matmuls (bridge engine-start -> first data ready)
BF16_W = False     # stationary (weights) in bf16, moving in f32r (cadence test)
TRACE = False      # set True to capture an NTFF profile (fills LAST_EXEC_NS)
LAST_EXEC_NS = None

_CACHE = {}

# x chunk row ranges (padded-row indices).  Block yb consumes rows [8yb, 8yb+10).
# Sized so each chunk's ring arrival (~2.8B/ns/partition from ~8.4us, +0.9us
# sem propagation) beats block yb's consumption time (~T0 + 2.15us * yb).
CHUNKS0 = [(0, 10), (10, 18), (18, 34), (34, PH)]   # img0, on sync
CHUNKS1 = [(0, 22), (22, 44), (44, PH)]             # img1, on scalar


def _build():
    wdt = BF16 if BF16_W else F32R
    nc = bacc.Bacc("TRN2", target_bir_lowering=False, debug=False, num_devices=NCORES)
    x_d = nc.dram_tensor("x", [BPC, CIN, XLEN], F32R, kind="ExternalInput").ap()
    w_d = nc.dram_tensor("w", [CIN, KK * KK * COUT], wdt, kind="ExternalInput").ap()
    o_d = nc.dram_tensor("o", [BPC, COUT, HW], F32, kind="ExternalOutput").ap()

    with tile.TileContext(nc) as tc:
        with (
            tc.tile_pool(name="wt", bufs=1) as wtp,
            tc.tile_pool(name="xin", bufs=2) as xp,
            tc.tile_pool(name="ps", bufs=6, space="PSUM") as pp,
            tc.tile_pool(name="ot", bufs=4) as op,
            tc.tile_pool(name="warm", bufs=1) as wmp,
            tc.tile_pool(name="warmps", bufs=1, space="PSUM") as wpp,
        ):
            # --- warmup: keep the PE busy from engine start until data lands.
            # memset on vector (earliest-starting engine, idle until the first
            # PSUM eviction at ~12us).
            wz = wmp.tile([CIN, 4 * COUT], BF16)
            nc.vector.memset(wz[:], 0.0)
            wps = wpp.tile([COUT, 4 * COUT], F32)
            for _ in range(WARMN):
                nc.tensor.matmul(wps[:], wz[:, :COUT], wz[:], start=True, stop=True)

            # --- input DMAs ---
            wt = wtp.tile([CIN, KK * KK * COUT], wdt)
            xins = []
            for lb in range(BPC):
                xin = xp.tile([CIN, XLEN], F32R, tag="xin")
                xins.append(xin)

            # img0 chunk 0 (rows 0-9) on sync: first-matmul dependency,
            # biggest transfer of the critical pair -> first in HWDGE order.
            r0, r1 = CHUNKS0[0]
            nc.sync.dma_start(xins[0][:, PW * r0 : PW * r1], x_d[0][:, PW * r0 : PW * r1])
            # w in 3 tap-groups on scalar (tap k is consumed at ~t0+0.24k us)
            for g in range(3):
                nc.scalar.dma_start(
                    wt[:, g * 3 * COUT : (g + 1) * 3 * COUT],
                    w_d[:, g * 3 * COUT : (g + 1) * 3 * COUT],
                )
            # img0 remaining chunks on sync
            for r0, r1 in CHUNKS0[1:]:
                nc.sync.dma_start(
                    xins[0][:, PW * r0 : PW * r1], x_d[0][:, PW * r0 : PW * r1]
                )
            # img1 on scalar (after w; needed only from ~27us)
            for r0, r1 in CHUNKS1:
                nc.scalar.dma_start(
                    xins[1][:, PW * r0 : PW * r1], x_d[1][:, PW * r0 : PW * r1]
                )

            # --- conv: 9 accumulating matmuls per 8-row block ---
            for lb in range(BPC):
                xrf = xins[lb][:].rearrange("p (r c) -> p r c", c=PW)  # [128,66,66]
                for yb in range(NBLK):
                    y0 = yb * ROWBLK
                    ps = pp.tile([COUT, ROWBLK * W], F32)
                    first = True
                    for dy in range(KK):
                        for dx in range(KK):
                            nc.tensor.matmul(
                                ps[:],
                                wt[:, (dy * KK + dx) * COUT : (dy * KK + dx + 1) * COUT],
                                xrf[:, y0 + dy : y0 + dy + ROWBLK, dx : dx + W],
                                start=first,
                                stop=(dy == KK - 1 and dx == KK - 1),
                            )
                            first = False
                    ot = op.tile([COUT, ROWBLK * W], F32)
                    if lb == BPC - 1 and yb == NBLK - 1:
                        # final block in halves, copies on two engines in
                        # parallel, so the kernel-exit drain starts sooner
                        hw2 = ROWBLK * W // 2
                        for h_, ceng, deng in (
                            (0, nc.vector, nc.sync),
                            (1, nc.scalar, nc.scalar),
                        ):
                            sl = slice(h_ * hw2, (h_ + 1) * hw2)
                            if ceng is nc.vector:
                                ceng.tensor_copy(ot[:, sl], ps[:, sl])
                            else:
                                ceng.copy(ot[:, sl], ps[:, sl])
                            deng.dma_start(
                                o_d[lb][:, W * y0 + h_ * hw2 : W * y0 + (h_ + 1) * hw2],
                                ot[:, sl],
                            )
                    else:
                        nc.vector.tensor_copy(ot[:], ps[:])
                        nc.scalar.dma_start(
                            o_d[lb][:, W * y0 : W * y0 + ROWBLK * W], ot[:]
                        )
    nc.compile()
    return nc


def _get_nc():
    key = ("nc_v3", BF16_W, WARMN)
    if key not in _CACHE:
        _CACHE[key] = _build()
    return _CACHE[key]


def _round_f32r(a):
    """RNE-round fp32 values to fp32r (keep top 20 bits: 1s+8e+11m)."""
    u = np.ascontiguousarray(a, dtype=np.float32).view(np.uint32)
    lsb = (u >> np.uint32(12)) & np.uint32(1)
    r = u + np.uint32(0x7FF) + lsb
    return (r & np.uint32(0xFFFFF000)).view(np.float32)


def kernel(x, weights):
    """x: [16,128,64,64] f32; weights: [128,128,9] f32 -> [2048,64,64] f32."""
    global LAST_EXEC_NS
    x = np.asarray(x, dtype=np.float32)
    w = np.asarray(weights, dtype=np.float32)
    # [cout, cin, k] -> [cin, k, cout] so tap k is a contiguous lhsT slice
    wT = np.ascontiguousarray(w.transpose(1, 2, 0)).reshape(CIN, KK * KK * COUT)
    if BF16_W:
        import ml_dtypes

        wT = wT.astype(ml_dtypes.bfloat16)
    else:
        wT = _round_f32r(wT)
    xpad = np.zeros((B, CIN, PH, PW), np.float32)
    xpad[:, :, 1 : H + 1, 1 : W + 1] = x
    xpad = _round_f32r(xpad.reshape(-1)).reshape(B, CIN, XLEN)

    nc = _get_nc()
    xr = xpad.reshape(NCORES, BPC, CIN, XLEN)
    in_maps = [{"x": np.ascontiguousarray(xr[c]), "w": wT} for c in range(NCORES)]

    res = bass_utils.run_bass_kernel_spmd(
        nc, in_maps, core_ids=list(range(NCORES)), trace=TRACE
    )
    LAST_EXEC_NS = res.exec_time_ns

    arr = np.stack([res.results[c]["o"] for c in range(NCORES)])  # [8, 2, 128, 4096]
    # out[cout*B + b] = conv[b, cout], with b = core*BPC + lb
    arr = arr.transpose(2, 0, 1, 3).reshape(COUT, B, H, W)
    return np.ascontiguousarray(arr.reshape(COUT * B, H, W))


# revision 11
# speedup vs baseline: 1.1907x; 1.1907x over previous
"""Trainium2 Bass kernel for nn_CustomConv2d: 3x3 conv, B=16, Cin=Cout=128, H=W=64.

Strategy (v5):
  - Data-parallel over batch: 8 NeuronCores x 2 images each; the (128,128,9)
    weight is replicated (host pre-transposes to [cin, tap, cout] so tap k is
    a contiguous [cin, cout] stationary-operand slice).
  - fp16 matmuls (1 cycle/row like fp32r, 10-bit mantissa ~ fp32r precision,
    half the DMA bytes and half the LDWEIGHTS time; PSUM accumulates fp32).
  - Per image the feature map lives in SBUF as a 66x66 zero-padded plane
    (host-prepadded => every DMA is contiguous per partition).
  - Conv = 9 accumulating PE matmuls per 8-row output block (contraction over
    Cin=128 on the partition dim).
  - DMA plan: the two HWDGE engines (sync=SP, scalar=Activation) each own a
    hardware ring; the 16 DMA queues round-robin across rings, so bandwidth
    splits evenly between rings whenever both are loaded.  All input DMAs are
    therefore issued in strict consumption order, alternating sync/scalar, so
    the earliest-needed bytes always lead both rings; output DMAs follow.
  - PE warm-up: HAM un-throttles the PE clock only after ~3us of sustained
    array activity and re-throttles after idle windows, so fp16 junk matmuls
    (zero tile memset on the otherwise-idle vector engine) bridge from engine
    start (~6.5us) to first-conv-data (~10us) with no PE gap.
  - Tail: final block in halves, copies on vector+scalar in parallel, DMAs on
    sync+scalar, so the exit drain starts as soon as possible.
"""

import numpy as np

import concourse.bass as bass  # noqa: F401  (registers bass types)
import concourse.tile as tile
import concourse.mybir as mybir
from concourse import bacc, bass_utils

F32 = mybir.dt.float32
F16 = mybir.dt.float16

B, CIN, COUT, KK, H, W = 16, 128, 128, 3, 64, 64
NCORES = 8
BPC = B // NCORES  # images per core
HW = H * W         # 4096
PW = W + 2         # padded row length (66)
PH = H + 2         # padded rows (66)
XLEN = PH * PW     # 4356
ROWBLK = 8         # output rows per PSUM block (8*64=512 = one fp32 PSUM bank)
NBLK = H // ROWBLK # 8 blocks per image

WARMN = 8          # warmup matmuls (bridge engine-start -> first data ready)
TRACE = False      # set True to capture an NTFF profile (fills LAST_EXEC_NS)
LAST_EXEC_NS = None

_CACHE = {}

# img0 x chunks (padded-row ranges), consumption-ordered; block yb needs rows
# [8yb, 8yb+10).  img1 in 3 coarser chunks (needed only from ~28us).
CHUNKS0 = [(0, 10), (10, 18), (18, 26), (26, 34), (34, 50), (50, PH)]
CHUNKS1 = [(0, 22), (22, 44), (44, PH)]


def _build():
    nc = bacc.Bacc("TRN2", target_bir_lowering=False, debug=False, num_devices=NCORES)
    x_d = nc.dram_tensor("x", [BPC, CIN, XLEN], F16, kind="ExternalInput").ap()
    w_d = nc.dram_tensor("w", [CIN, KK * KK * COUT], F16, kind="ExternalInput").ap()
    o_d = nc.dram_tensor("o", [BPC, COUT, HW], F32, kind="ExternalOutput").ap()

    with tile.TileContext(nc) as tc:
        with (
            tc.tile_pool(name="wt", bufs=1) as wtp,
            tc.tile_pool(name="xin", bufs=2) as xp,
            tc.tile_pool(name="ps", bufs=6, space="PSUM") as pp,
            tc.tile_pool(name="ot", bufs=4) as op,
            tc.tile_pool(name="warm", bufs=1) as wmp,
            tc.tile_pool(name="warmps", bufs=1, space="PSUM") as wpp,
        ):
            # --- warmup: keep the PE busy from engine start until data lands.
            wz = wmp.tile([CIN, 4 * COUT], F16)
            nc.vector.memset(wz[:], 0.0)
            wps = wpp.tile([COUT, 4 * COUT], F32)
            for _ in range(WARMN):
                nc.tensor.matmul(wps[:], wz[:, :COUT], wz[:], start=True, stop=True)

            # --- input DMAs: strict consumption order, alternating the two
            # HWDGE rings so the earliest-needed bytes lead both rings.
            wt = wtp.tile([CIN, KK * KK * COUT], F16)
            xins = []
            for lb in range(BPC):
                xin = xp.tile([CIN, XLEN], F16, tag="xin")
                xins.append(xin)

            ring = [nc.sync, nc.scalar]
            plan = []
            plan.append((wt[:, : 3 * COUT], w_d[:, : 3 * COUT]))
            r0, r1 = CHUNKS0[0]
            plan.append((xins[0][:, PW * r0 : PW * r1], x_d[0][:, PW * r0 : PW * r1]))
            plan.append((wt[:, 3 * COUT : 6 * COUT], w_d[:, 3 * COUT : 6 * COUT]))
            plan.append((wt[:, 6 * COUT :], w_d[:, 6 * COUT :]))
            for r0, r1 in CHUNKS0[1:]:
                plan.append(
                    (xins[0][:, PW * r0 : PW * r1], x_d[0][:, PW * r0 : PW * r1])
                )
            for r0, r1 in CHUNKS1:
                plan.append(
                    (xins[1][:, PW * r0 : PW * r1], x_d[1][:, PW * r0 : PW * r1])
                )
            for i, (dst, src) in enumerate(plan):
                ring[i % 2].dma_start(dst, src)

            # --- conv: 9 accumulating matmuls per 8-row block ---
            nout = 0
            for lb in range(BPC):
                xrf = xins[lb][:].rearrange("p (r c) -> p r c", c=PW)  # [128,66,66]
                for yb in range(NBLK):
                    y0 = yb * ROWBLK
                    ps = pp.tile([COUT, ROWBLK * W], F32)
                    first = True
                    for dy in range(KK):
                        for dx in range(KK):
                            nc.tensor.matmul(
                                ps[:],
                                wt[:, (dy * KK + dx) * COUT : (dy * KK + dx + 1) * COUT],
                                xrf[:, y0 + dy : y0 + dy + ROWBLK, dx : dx + W],
                                start=first,
                                stop=(dy == KK - 1 and dx == KK - 1),
                            )
                            first = False
                    ot = op.tile([COUT, ROWBLK * W], F32)
                    if lb == BPC - 1 and yb == NBLK - 1:
                        # final block in halves, copies on two engines in
                        # parallel, so the kernel-exit drain starts sooner
                        hw2 = ROWBLK * W // 2
                        for h_, deng in ((0, nc.sync), (1, nc.scalar)):
                            sl = slice(h_ * hw2, (h_ + 1) * hw2)
                            if h_ == 0:
                                nc.vector.tensor_copy(ot[:, sl], ps[:, sl])
                            else:
                                nc.scalar.copy(ot[:, sl], ps[:, sl])
                            deng.dma_start(
                                o_d[lb][:, W * y0 + h_ * hw2 : W * y0 + (h_ + 1) * hw2],
                                ot[:, sl],
                            )
                    else:
                        nc.vector.tensor_copy(ot[:], ps[:])
                        # alternate output rings too (inputs are all queued
                        # ahead of every output on both engines)
                        ring[nout % 2].dma_start(
                            o_d[lb][:, W * y0 : W * y0 + ROWBLK * W], ot[:]
                        )
                        nout += 1
    nc.compile()
    return nc


def _get_nc():
    key = ("nc_v5", WARMN)
    if key not in _CACHE:
        _CACHE[key] = _build()
    return _CACHE[key]


def kernel(x, weights):
    """x: [16,128,64,64] f32; weights: [128,128,9] f32 -> [2048,64,64] f32."""
    global LAST_EXEC_NS
    x = np.asarray(x, dtype=np.float32)
    w = np.asarray(weights, dtype=np.float32)
    # [cout, cin, k] -> [cin, k, cout] so tap k is a contiguous lhsT slice
    wT = np.ascontiguousarray(w.transpose(1, 2, 0)).reshape(CIN, KK * KK * COUT)
    wT = wT.astype(np.float16)
    xpad = np.zeros((B, CIN, PH, PW), np.float16)
    xpad[:, :, 1 : H + 1, 1 : W + 1] = x.astype(np.float16)
    xpad = xpad.reshape(B, CIN, XLEN)

    nc = _get_nc()
    xr = xpad.reshape(NCORES, BPC, CIN, XLEN)
    in_maps = [{"x": np.ascontiguousarray(xr[c]), "w": wT} for c in range(NCORES)]

    res = bass_utils.run_bass_kernel_spmd(
        nc, in_maps, core_ids=list(range(NCORES)), trace=TRACE
    )
    LAST_EXEC_NS = res.exec_time_ns

    arr = np.stack([res.results[c]["o"] for c in range(NCORES)])  # [8, 2, 128, 4096]
    # out[cout*B + b] = conv[b, cout], with b = core*BPC + lb
    arr = arr.transpose(2, 0, 1, 3).reshape(COUT, B, H, W)
    return np.ascontiguousarray(arr.reshape(COUT * B, H, W))
